# revision 11
# baseline (speedup 1.0000x reference)
"""Trainium2 Bass kernel for nn_GNOME_42588895707869 (GNN message passing + cdist).

Sharding: cores 0-3 process graph 1, cores 4-7 process graph 2. Within a quad,
core q owns dst-nodes [2048q, 2048(q+1)). Edges are partitioned by dst, sorted,
grouped into 16 windows of 128 nodes, padded to 128-edge chunks. Message
passing per layer: dma_gather of x rows from HBM, relu(x[src]+e) in bf16,
one-hot matmul segment-sum into PSUM (fp32 accumulate), node MLP in fp32r,
quad AllGather of updated x rows. Final cdist: 8-core AllGather of
feature-major m slices; each core's 1024-row block is selected via dma_gather
with per-core index data (keeps the SPMD program core-uniform).
"""
import sys

sys.path.insert(0, "/opt/trn_rl_repo")

import numpy as np  # noqa: E402

N = 8192
H = 128
L = 6
CAT = 768
E = 131072
NQ = 2048          # nodes per core
NWIN = 16          # 128-node windows per core
WSZ = 128
XIN = 64           # features(48) + RW(16)
ROWS_D = 1024      # cdist rows per core
EPS = 1e-12


# ---------------------------------------------------------------- host prep
def _pack_graph(edge_index, e_features):
    src = np.asarray(edge_index[0]).astype(np.int64)
    dst = np.asarray(edge_index[1]).astype(np.int64)
    ef = np.asarray(e_features, dtype=np.float32)
    cores = []
    wchunks = []
    for q in range(4):
        m = (dst // NQ) == q
        s_q, d_q, ef_q = src[m], dst[m] - q * NQ, ef[m]
        order = np.argsort(d_q, kind="stable")
        s_q, d_q, ef_q = s_q[order], d_q[order], ef_q[order]
        cnt = np.bincount(d_q // WSZ, minlength=NWIN)
        cores.append((s_q, d_q, ef_q, cnt))
        wchunks.append(int(np.ceil(cnt / 128).max()))
    return cores, max(wchunks)


def _grid_layout(core, wc):
    s_q, d_q, ef_q, cnt = core
    C = NWIN * wc
    src_idx = np.zeros(C * 128, dtype=np.int16)
    dstrel = np.full(C * 128, -1.0, dtype=np.float32)
    ef_perm = np.zeros((C * 128, 9), dtype=np.float32)
    off = np.concatenate([[0], np.cumsum(cnt)])
    for w in range(NWIN):
        a, b = int(off[w]), int(off[w + 1])
        n = b - a
        base = w * wc * 128
        src_idx[base:base + n] = s_q[a:b]
        dstrel[base:base + n] = (d_q[a:b] - w * WSZ).astype(np.float32)
        ef_perm[base:base + n, :8] = ef_q[a:b]
        ef_perm[base:base + n, 8] = 1.0
    return src_idx, dstrel, ef_perm


def _idx_sb(idx):
    n = idx.shape[0]
    assert n % 16 == 0
    a = np.ascontiguousarray(idx.astype(np.int16).reshape(n // 16, 16).T)
    return np.tile(a, (8, 1)).copy()


# ---------------------------------------------------------------- program
_prog_cache = {}


def _build_program(C):
    import concourse.bass as bass  # noqa: F401
    import concourse.mybir as mybir
    from concourse import bacc
    from concourse.tile import TileContext
    from concourse.masks import make_identity

    f32 = mybir.dt.float32
    f32r = mybir.dt.float32r
    bf16 = mybir.dt.bfloat16
    i16 = mybir.dt.int16
    AF = mybir.ActivationFunctionType
    Alu = mybir.AluOpType

    WC = C // NWIN
    SLAB = 16                      # gather slab, chunks
    assert C % SLAB == 0

    nc = bacc.Bacc("TRN2", num_devices=8)

    xin = nc.declare_dram_parameter("xin", [XIN + 1, NQ], f32, isOutput=False)
    wpre = nc.declare_dram_parameter("wpre", [XIN + 1, H], f32, isOutput=False)
    wedge = nc.declare_dram_parameter("wedge", [9, H], f32, isOutput=False)
    efT = nc.declare_dram_parameter("efT", [9, C * 128], f32, isOutput=False)
    srci = nc.declare_dram_parameter("srci", [128, C * 8], i16, isOutput=False)
    dstrel_d = nc.declare_dram_parameter("dstrel", [128, C], f32, isOutput=False)
    gw1 = nc.declare_dram_parameter("gw1", [L, H, H], f32, isOutput=False)
    gw2 = nc.declare_dram_parameter("gw2", [L, H, H], f32, isOutput=False)
    gb1t = nc.declare_dram_parameter("gb1t", [H, L], f32, isOutput=False)
    gb2t = nc.declare_dram_parameter("gb2t", [H, L], f32, isOutput=False)
    wo1 = nc.declare_dram_parameter("wo1", [CAT, 2 * CAT], f32, isOutput=False)
    wo2 = nc.declare_dram_parameter("wo2", [2 * CAT, CAT], f32, isOutput=False)
    bo1t = nc.declare_dram_parameter("bo1t", [H, 12], f32, isOutput=False)
    bo2t = nc.declare_dram_parameter("bo2t", [H, 6], f32, isOutput=False)
    m1i = nc.declare_dram_parameter("m1i", [128, 48], i16, isOutput=False)
    nsqi = nc.declare_dram_parameter("nsqi", [128, 8], i16, isOutput=False)
    out = nc.declare_dram_parameter("out", [ROWS_D, N], f32, isOutput=True)

    x_rows = nc.dram_tensor("x_rows", [N, H], f32)
    x_ag_in = nc.dram_tensor("x_ag_in", [NQ, H], f32)
    outs_hbm = nc.dram_tensor("outs_hbm", [L, H, NQ], f32r)
    mslice = nc.dram_tensor("mslice", [CAT + 1, NQ], f32)
    mT_all = nc.dram_tensor("mT_all", [8 * (CAT + 1), NQ], f32, addr_space="Shared")

    quads = [[0, 1, 2, 3], [4, 5, 6, 7]]
    allg = [[0, 1, 2, 3, 4, 5, 6, 7]]

    with TileContext(nc) as tc:
        cpool = tc.alloc_tile_pool(name="const", bufs=1)
        ident = cpool.tile([128, 128], f32)
        make_identity(nc, ident[:])
        identr = cpool.tile([128, 128], f32r)
        nc.vector.tensor_copy(identr[:], ident[:])
        iota = cpool.tile([128, WSZ], f32)
        nc.gpsimd.iota(iota[:], pattern=[[1, WSZ]], base=0,
                       channel_multiplier=0,
                       allow_small_or_imprecise_dtypes=True)
        gb1s = cpool.tile([H, L], f32)
        nc.sync.dma_start(out=gb1s[:], in_=gb1t[:])
        gb2s = cpool.tile([H, L], f32)
        nc.sync.dma_start(out=gb2s[:], in_=gb2t[:])
        w1r = cpool.tile([H, L, H], f32r)
        w2r = cpool.tile([H, L, H], f32r)
        xcur = cpool.tile([H, NQ], f32r)
        feat_t = cpool.tile([H, NQ], f32r)
        gpool = tc.alloc_tile_pool(name="grid", bufs=1)
        srct = gpool.tile([128, C * 8], i16)
        nc.sync.dma_start(out=srct[:], in_=srci[:])
        dstrel_t = gpool.tile([128, C], f32)
        nc.sync.dma_start(out=dstrel_t[:], in_=dstrel_d[:])
        e_grid = gpool.tile([128, C, H], bf16)
        aggT = gpool.tile([H, NQ], f32)

        # ---------------- phase A ------------------------------------
        with tc.tile_pool(name="phA", bufs=2) as pa, \
             tc.tile_pool(name="psA", bufs=2, space="PSUM") as ppa:
            wtmp = pa.tile([H, L, H], f32, tag="wtmp")
            nc.sync.dma_start(out=wtmp[:], in_=gw1[:].rearrange("l k m -> k l m"))
            nc.vector.tensor_copy(w1r[:], wtmp[:])
            wtmp2 = pa.tile([H, L, H], f32, tag="wtmp")
            nc.sync.dma_start(out=wtmp2[:], in_=gw2[:].rearrange("l k m -> k l m"))
            nc.vector.tensor_copy(w2r[:], wtmp2[:])

            xinf = pa.tile([XIN + 1, NQ], f32, tag="xinf")
            nc.sync.dma_start(out=xinf[:], in_=xin[:])
            xinr = pa.tile([XIN + 1, NQ], f32r, tag="xinr")
            nc.vector.tensor_copy(xinr[:], xinf[:])
            wpref = pa.tile([XIN + 1, H], f32, tag="wpref")
            nc.sync.dma_start(out=wpref[:], in_=wpre[:])
            wprer = pa.tile([XIN + 1, H], f32r, tag="wprer")
            nc.vector.tensor_copy(wprer[:], wpref[:])
            for nt in range(NQ // 512):
                ps = ppa.tile([H, 512], f32, space="PSUM", tag="psx")
                nc.tensor.matmul(ps[:], lhsT=wprer[:],
                                 rhs=xinr[:, nt * 512:(nt + 1) * 512],
                                 start=True, stop=True)
                nc.vector.tensor_copy(xcur[:, nt * 512:(nt + 1) * 512], ps[:])
            nc.vector.tensor_copy(feat_t[:], xcur[:])

            # x0 rows -> x_ag_in -> AG -> x_rows
            for t in range(NQ // 128):
                pst = ppa.tile([128, 128], f32r, space="PSUM", tag="pst")
                nc.tensor.transpose(out=pst[:],
                                    in_=xcur[:, t * 128:(t + 1) * 128],
                                    identity=identr[:])
                xr = pa.tile([128, H], f32, tag="xr")
                nc.scalar.activation(xr[:], pst[:], AF.Copy)
                nc.sync.dma_start(
                    out=x_ag_in[:].rearrange("(a p) m -> a p m", p=128)[t],
                    in_=xr[:])
            nc.gpsimd.collective_compute(
                "AllGather", Alu.bypass, ins=[x_ag_in[:]], outs=[x_rows[:]],
                replica_groups=quads)

            # edge MLP -> e_grid bf16
            weg = pa.tile([9, H], f32, tag="weg")
            nc.sync.dma_start(out=weg[:], in_=wedge[:])
            wegb = pa.tile([9, H], bf16, tag="wegb")
            nc.vector.tensor_copy(wegb[:], weg[:])
            ES = 16
            for sl in range((C + ES - 1) // ES):
                c0 = sl * ES
                cn = min(ES, C - c0)
                eslab = pa.tile([9, ES * 128], f32, tag="eslab")
                nc.sync.dma_start(out=eslab[:, :cn * 128],
                                  in_=efT[:, c0 * 128:(c0 + cn) * 128])
                eslabb = pa.tile([9, ES * 128], bf16, tag="eslabb")
                nc.vector.tensor_copy(eslabb[:, :cn * 128], eslab[:, :cn * 128])
                for c in range(cn):
                    pse = ppa.tile([128, H], f32, space="PSUM", tag="pse")
                    nc.tensor.matmul(pse[:],
                                     lhsT=eslabb[:, c * 128:(c + 1) * 128],
                                     rhs=wegb[:], start=True, stop=True)
                    nc.scalar.activation(e_grid[:, c0 + c, :], pse[:], AF.Copy)

        # ---------------- phase B: 6 GNN layers ----------------------
        with tc.tile_pool(name="phB", bufs=2) as pb, \
             tc.tile_pool(name="ohB", bufs=3) as pob, \
             tc.tile_pool(name="psB", bufs=2, space="PSUM") as ppb:
            ppm = ppb
            for l in range(L):
                # --- message pass + segment sum
                for w in range(NWIN):
                    psagg = ppb.tile([128, H], f32, space="PSUM", tag="psagg")
                    for cw in range(WC):
                        c = w * WC + cw
                        if c % SLAB == 0:
                            xg = pb.tile([128, SLAB, H], f32, tag="xg")
                            nc.gpsimd.dma_gather(
                                xg[:], x_rows[:],
                                srct[:, c * 8:c * 8 + SLAB * 8],
                                SLAB * 128, SLAB * 128, H, elem_step=H,
                                single_packet=False)
                            tmp = pb.tile([128, SLAB, H], bf16, tag="tmpadd")
                            nc.vector.tensor_tensor(
                                out=tmp[:], in0=xg[:],
                                in1=e_grid[:, c:c + SLAB, :], op=Alu.add)
                            msg = pb.tile([128, SLAB, H], bf16, tag="msg")
                            nc.scalar.activation(msg[:], tmp[:], AF.Relu)
                            oh = pob.tile([128, SLAB, WSZ], bf16, tag="oh")
                            nc.vector.tensor_tensor(
                                out=oh[:],
                                in0=dstrel_t[:, c:c + SLAB].to_broadcast(
                                    [128, SLAB, WSZ]),
                                in1=iota[:].rearrange("p n -> p () n").broadcast_to(
                                    [128, SLAB, WSZ]),
                                op=Alu.is_equal)
                        sc = c % SLAB
                        nc.tensor.matmul(psagg[:], lhsT=oh[:, sc, :],
                                         rhs=msg[:, sc, :],
                                         start=(cw == 0), stop=(cw == WC - 1))
                    # window epilogue: psum [128 nodes, H] -> aggT columns
                    aggn = pb.tile([128, H], f32, tag="aggn")
                    nc.scalar.activation(aggn[:], psagg[:], AF.Copy)
                    psT = ppm.tile([128, 128], f32, space="PSUM", tag="psT")
                    nc.tensor.transpose(out=psT[:], in_=aggn[:], identity=ident[:])
                    nc.vector.tensor_copy(aggT[:, w * WSZ:(w + 1) * WSZ], psT[:])

                # --- node MLP (own 2048 nodes)
                for nt in range(NQ // 512):
                    sl_ = slice(nt * 512, (nt + 1) * 512)
                    ht = pb.tile([H, 512], f32r, tag="ht")
                    nc.vector.tensor_tensor(out=ht[:], in0=xcur[:, sl_],
                                            in1=aggT[:, sl_], op=Alu.add)
                    ps1 = ppm.tile([H, 512], f32, space="PSUM", tag="psmlp")
                    nc.tensor.matmul(ps1[:], lhsT=w1r[:, l, :], rhs=ht[:],
                                     start=True, stop=True)
                    t1 = pb.tile([H, 512], f32r, tag="t1")
                    nc.scalar.activation(t1[:], ps1[:], AF.Relu,
                                         bias=gb1s[:, l:l + 1])
                    ps2 = ppm.tile([H, 512], f32, space="PSUM", tag="psmlp")
                    nc.tensor.matmul(ps2[:], lhsT=w2r[:, l, :], rhs=t1[:],
                                     start=True, stop=True)
                    if l in (1, 3):
                        s0 = pb.tile([H, 512], f32, space="SBUF", tag="s0")
                        nc.scalar.activation(s0[:], ps2[:], AF.Identity,
                                             bias=gb2s[:, l:l + 1])
                        nc.vector.tensor_tensor(out=feat_t[:, sl_], in0=s0[:],
                                                in1=feat_t[:, sl_], op=Alu.add)
                        nc.vector.tensor_relu(xcur[:, sl_], feat_t[:, sl_])
                    else:
                        nc.scalar.activation(xcur[:, sl_], ps2[:], AF.Relu,
                                             bias=gb2s[:, l:l + 1])
                # save layer output (f32r bytes) for phase C
                nc.sync.dma_start(out=outs_hbm[l], in_=xcur[:])
                # x rows AG for next layer
                if l < L - 1:
                    for t in range(NQ // 128):
                        pst = ppm.tile([128, 128], f32r, space="PSUM", tag="psT")
                        nc.tensor.transpose(out=pst[:],
                                            in_=xcur[:, t * 128:(t + 1) * 128],
                                            identity=identr[:])
                        xr = pb.tile([128, H], f32, tag="xr")
                        nc.scalar.activation(xr[:], pst[:], AF.Copy)
                        nc.sync.dma_start(
                            out=x_ag_in[:].rearrange("(a p) m -> a p m", p=128)[t],
                            in_=xr[:])
                    nc.gpsimd.collective_compute(
                        "AllGather", Alu.bypass, ins=[x_ag_in[:]],
                        outs=[x_rows[:]], replica_groups=quads)

        gpool.release()

        # ---------------- phase C: output MLP ------------------------
        with tc.tile_pool(name="phCw", bufs=1) as pcw, \
             tc.tile_pool(name="phC", bufs=2) as pc, \
             tc.tile_pool(name="phCh", bufs=1) as pch, \
             tc.tile_pool(name="psC", bufs=4, space="PSUM") as ppc:
            wo1r = pcw.tile([128, 6, 2 * CAT], f32r, tag="wo1r")
            wo2r = pcw.tile([128, 12, CAT], f32r, tag="wo2r")
            for kc in range(6):
                wt = pc.tile([128, 2 * CAT], f32, tag="wldtmp")
                nc.sync.dma_start(
                    out=wt[:],
                    in_=wo1[:].rearrange("(a p) m -> a p m", p=128)[kc])
                nc.vector.tensor_copy(wo1r[:, kc, :], wt[:])
            for kc in range(12):
                wt = pc.tile([128, CAT], f32, tag="wldtmp")
                nc.sync.dma_start(
                    out=wt[:],
                    in_=wo2[:].rearrange("(a p) m -> a p m", p=128)[kc])
                nc.vector.tensor_copy(wo2r[:, kc, :], wt[:])
            bo1s = pcw.tile([H, 12], f32, tag="bo1s")
            nc.sync.dma_start(out=bo1s[:], in_=bo1t[:])
            bo2s = pcw.tile([H, 6], f32, tag="bo2s")
            nc.sync.dma_start(out=bo2s[:], in_=bo2t[:])
            ones_r = pcw.tile([128, 1], f32r, tag="ones_r")
            onesf = pcw.tile([128, 1], f32, tag="onesf")
            nc.vector.memset(onesf[:], 1.0)
            nc.vector.tensor_copy(ones_r[:], onesf[:])

            nsq_sb = pcw.tile([1, NQ], f32, tag="nsq_sb")
            for nt in range(NQ // 512):
                sl_ = slice(nt * 512, (nt + 1) * 512)
                ne_t = []
                for kc in range(6):
                    nt_t = pc.tile([H, 512], f32r, tag=f"ne{kc}")
                    nc.sync.dma_start(out=nt_t[:], in_=outs_hbm[kc][:, sl_])
                    ne_t.append(nt_t)
                h1 = pch.tile([128, 12, 512], f32r, tag="h1")
                for mt in range(12):
                    ps = ppc.tile([128, 512], f32, space="PSUM", tag="psc")
                    for kc in range(6):
                        nc.tensor.matmul(
                            ps[:], lhsT=wo1r[:, kc, mt * 128:(mt + 1) * 128],
                            rhs=ne_t[kc][:], start=(kc == 0), stop=(kc == 5))
                    nc.scalar.activation(h1[:, mt, :], ps[:], AF.Relu,
                                         bias=bo1s[:, mt:mt + 1])
                sqsum = ppc.tile([1, 512], f32, space="PSUM", tag="sqsum")
                for m2 in range(6):
                    ps = ppc.tile([128, 512], f32, space="PSUM", tag="psc")
                    for kc in range(12):
                        nc.tensor.matmul(
                            ps[:], lhsT=wo2r[:, kc, m2 * 128:(m2 + 1) * 128],
                            rhs=h1[:, kc, :], start=(kc == 0), stop=(kc == 11))
                    mtile = pc.tile([128, 512], f32, tag="mtile")
                    nc.scalar.activation(mtile[:], ps[:], AF.Identity,
                                         bias=bo2s[:, m2:m2 + 1])
                    nc.sync.dma_start(
                        out=mslice[m2 * 128:(m2 + 1) * 128, sl_], in_=mtile[:])
                    sq = pc.tile([128, 512], f32r, tag="sq")
                    nc.vector.tensor_tensor(out=sq[:], in0=mtile[:],
                                            in1=mtile[:], op=Alu.mult)
                    nc.tensor.matmul(sqsum[:], lhsT=ones_r[:], rhs=sq[:],
                                     start=(m2 == 0), stop=(m2 == 5))
                nc.vector.tensor_copy(nsq_sb[:, sl_], sqsum[:])
            nc.sync.dma_start(out=mslice[CAT:CAT + 1, :], in_=nsq_sb[:])
            nc.gpsimd.collective_compute(
                "AllGather", Alu.bypass, ins=[mslice[:]], outs=[mT_all[:]],
                replica_groups=allg)

        # ---------------- phase D: cdist -----------------------------
        with tc.tile_pool(name="phD1", bufs=1) as pd1, \
             tc.tile_pool(name="phD", bufs=2) as pd, \
             tc.tile_pool(name="ohD", bufs=3) as pdd, \
             tc.tile_pool(name="psD", bufs=4, space="PSUM") as ppd:
            m1it = pd1.tile([128, 48], i16, tag="m1it")
            nc.sync.dma_start(out=m1it[:], in_=m1i[:])
            nsqit = pd1.tile([128, 8], i16, tag="nsqit")
            nc.sync.dma_start(out=nsqit[:], in_=nsqi[:])
            vtab = mT_all[:].rearrange("a (b c) -> (a b) c", c=1024)
            m1pre = pd1.tile([128, 6, 1024], f32, tag="m1pre")
            nc.gpsimd.dma_gather(m1pre[:], vtab, m1it[:], CAT, CAT, 1024,
                                 elem_step=1024, single_packet=False)
            m1r = pd1.tile([128, 6, 1024], f32r, tag="m1r")
            nc.vector.tensor_scalar_mul(m1r[:], m1pre[:], -2.0)
            n1all = pd1.tile([128, 1, 1024], f32, tag="n1all")
            nc.gpsimd.dma_gather(n1all[:], vtab, nsqit[:], 128, 128, 1024,
                                 elem_step=1024, single_packet=False)
            n1b = []
            epsb = []
            for b in range(8):
                psn = ppd.tile([128, 128], f32, space="PSUM", tag="psn")
                nc.tensor.transpose(out=psn[:],
                                    in_=n1all[:, 0, b * 128:(b + 1) * 128],
                                    identity=ident[:])
                nb = pd1.tile([128, 1], f32, tag=f"n1b{b}")
                nc.vector.tensor_copy(nb[:], psn[:, 0:1])
                eb = pd1.tile([128, 1], f32, tag=f"epsb{b}")
                nc.vector.tensor_scalar(out=eb[:], in0=nb[:], scalar1=-1.0,
                                        scalar2=EPS, op0=Alu.mult, op1=Alu.add)
                n1b.append(nb)
                epsb.append(eb)
            ones1f = pd1.tile([1, 128], f32, tag="ones1f")
            nc.vector.memset(ones1f[:], 1.0)
            ones1 = pd1.tile([1, 128], f32r, tag="ones1")
            nc.vector.tensor_copy(ones1[:], ones1f[:])

            for s in range(16):
                qs, soff = s // 4, (s % 4) * 512
                base = (4 + qs) * (CAT + 1)
                stf = pd.tile([128, 6, 512], f32, tag="stf")
                nc.sync.dma_start(
                    out=stf[:],
                    in_=mT_all[base:base + CAT, soff:soff + 512].rearrange(
                        "(a p) m -> p a m", p=128))
                st_r = pd.tile([128, 6, 512], f32r, tag="st_r")
                nc.vector.tensor_copy(st_r[:], stf[:])
                n2f = pd.tile([1, 512], f32, tag="n2f")
                nc.sync.dma_start(out=n2f[:],
                                  in_=mT_all[base + CAT:base + CAT + 1,
                                             soff:soff + 512])
                n2r = pd.tile([1, 512], f32r, tag="n2r")
                nc.vector.tensor_copy(n2r[:], n2f[:])
                for b in range(8):
                    psd = ppd.tile([128, 512], f32, space="PSUM", tag="psd")
                    for kc in range(6):
                        nc.tensor.matmul(psd[:],
                                         lhsT=m1r[:, kc, b * 128:(b + 1) * 128],
                                         rhs=st_r[:, kc, :],
                                         start=(kc == 0), stop=False)
                    nc.tensor.matmul(psd[:], lhsT=ones1[:], rhs=n2r[:],
                                     start=False, stop=True)
                    s1 = pdd.tile([128, 512], f32, tag="s1")
                    nc.vector.tensor_scalar(out=s1[:], in0=psd[:],
                                            scalar1=epsb[b][:], scalar2=0.0,
                                            op0=Alu.max, op1=Alu.add)
                    dt_ = pdd.tile([128, 512], f32, tag="dt_")
                    nc.scalar.activation(dt_[:], s1[:], AF.Sqrt,
                                         bias=n1b[b][:])
                    nc.sync.dma_start(
                        out=out[b * 128:(b + 1) * 128, s * 512:(s + 1) * 512],
                        in_=dt_[:])
        cpool.release()

    nc.compile()
    return nc


# ---------------------------------------------------------------- entry
def kernel(**inputs):
    from concourse.bass_utils import run_bass_kernel_spmd

    g1, wc1 = _pack_graph(inputs["edge_index_1"], inputs["e_features1"])
    g2, wc2 = _pack_graph(inputs["edge_index_2"], inputs["e_features2"])
    wc = max(wc1, wc2)
    C = NWIN * wc
    if C % 16 != 0:
        wc += (-wc) % 1  # SLAB=16 divides C iff (16*wc)%16==0, always true
    C = NWIN * wc

    feats = [np.asarray(inputs["features_1"], dtype=np.float32),
             np.asarray(inputs["features_2"], dtype=np.float32)]
    rws = [np.asarray(inputs["RW_1"], dtype=np.float32),
           np.asarray(inputs["RW_2"], dtype=np.float32)]

    wpre_aug = np.vstack([np.asarray(inputs["W_pre"], dtype=np.float32),
                          np.asarray(inputs["b_pre"], dtype=np.float32)[None]])
    wedge_aug = np.vstack([np.asarray(inputs["W_edge"], dtype=np.float32),
                           np.asarray(inputs["b_edge"], dtype=np.float32)[None]])
    gw1 = np.asarray(inputs["gnn_w1"], dtype=np.float32)
    gw2 = np.asarray(inputs["gnn_w2"], dtype=np.float32)
    gb1t = np.ascontiguousarray(np.asarray(inputs["gnn_b1"], np.float32).T)
    gb2t = np.ascontiguousarray(np.asarray(inputs["gnn_b2"], np.float32).T)
    wo1 = np.asarray(inputs["W_out1"], dtype=np.float32)
    wo2 = np.asarray(inputs["W_out2"], dtype=np.float32)
    bo1t = np.ascontiguousarray(
        np.asarray(inputs["b_out1"], np.float32).reshape(12, 128).T)
    bo2t = np.ascontiguousarray(
        np.asarray(inputs["b_out2"], np.float32).reshape(6, 128).T)

    in_maps = []
    for k in range(8):
        g = k // 4          # graph id
        q = k % 4           # quad rank
        src_idx, dstrel, ef_perm = _grid_layout((g1 if g == 0 else g2)[q], wc)
        fx = feats[g][q * NQ:(q + 1) * NQ]
        rx = rws[g][q * NQ:(q + 1) * NQ]
        xin = np.concatenate(
            [fx, rx, np.ones((NQ, 1), np.float32)], axis=1).T.copy()
        # phase D row-block selection: core k -> m1 rows [1024k, 1024k+1024)
        qq, hh = k // 2, k % 2
        m1idx = (2 * ((CAT + 1) * qq + np.arange(CAT)) + hh).astype(np.int16)
        nsqidx = np.full(128, 2 * ((CAT + 1) * qq + CAT) + hh, dtype=np.int16)
        in_maps.append({
            "xin": np.ascontiguousarray(xin),
            "wpre": wpre_aug, "wedge": wedge_aug,
            "efT": np.ascontiguousarray(ef_perm.T),
            "srci": _idx_sb(src_idx),
            "dstrel": np.ascontiguousarray(dstrel.reshape(C, 128).T),
            "gw1": gw1, "gw2": gw2, "gb1t": gb1t, "gb2t": gb2t,
            "wo1": wo1, "wo2": wo2, "bo1t": bo1t, "bo2t": bo2t,
            "m1i": _idx_sb(m1idx),
            "nsqi": _idx_sb(nsqidx),
        })

    if C not in _prog_cache:
        _prog_cache[C] = _build_program(C)
    nc = _prog_cache[C]
    res = run_bass_kernel_spmd(nc, in_maps, list(range(8)))
    return np.vstack([np.asarray(res.results[k]["out"]) for k in range(8)])


# revision 12
# speedup vs baseline: 3215.1981x; 3215.1981x over previous
"""Trainium2 Bass kernel for nn_GNOME_42588895707869 (GNN message passing + cdist).

Sharding: cores 0-3 process graph 1, cores 4-7 process graph 2. Within a quad,
core q owns dst-nodes [2048q, 2048(q+1)). Edges are partitioned by dst, sorted,
grouped into 16 windows of 128 nodes, padded to 128-edge chunks. Message
passing per layer: dma_gather of x rows from HBM, relu(x[src]+e) in bf16,
one-hot matmul segment-sum into PSUM (fp32 accumulate), node MLP in fp32r,
quad AllGather of updated x rows. Final cdist: 8-core AllGather of
feature-major m slices; each core's 1024-row block is selected via dma_gather
with per-core index data (keeps the SPMD program core-uniform).
"""
import sys

sys.path.insert(0, "/opt/trn_rl_repo")

import numpy as np  # noqa: E402

N = 8192
H = 128
L = 6
CAT = 768
E = 131072
NQ = 2048          # nodes per core
NWIN = 16          # 128-node windows per core
WSZ = 128
XIN = 64           # features(48) + RW(16)
ROWS_D = 1024      # cdist rows per core
EPS = 1e-12


# ---------------------------------------------------------------- host prep
def _pack_graph(edge_index, e_features):
    src = np.asarray(edge_index[0]).astype(np.int64)
    dst = np.asarray(edge_index[1]).astype(np.int64)
    ef = np.asarray(e_features, dtype=np.float32)
    cores = []
    wchunks = []
    for q in range(4):
        m = (dst // NQ) == q
        s_q, d_q, ef_q = src[m], dst[m] - q * NQ, ef[m]
        order = np.argsort(d_q, kind="stable")
        s_q, d_q, ef_q = s_q[order], d_q[order], ef_q[order]
        cnt = np.bincount(d_q // WSZ, minlength=NWIN)
        cores.append((s_q, d_q, ef_q, cnt))
        wchunks.append(int(np.ceil(cnt / 128).max()))
    return cores, max(wchunks)


def _grid_layout(core, wc):
    s_q, d_q, ef_q, cnt = core
    C = NWIN * wc
    src_idx = np.zeros(C * 128, dtype=np.int16)
    dstrel = np.full(C * 128, -1.0, dtype=np.float32)
    ef_perm = np.zeros((C * 128, 9), dtype=np.float32)
    off = np.concatenate([[0], np.cumsum(cnt)])
    for w in range(NWIN):
        a, b = int(off[w]), int(off[w + 1])
        n = b - a
        base = w * wc * 128
        src_idx[base:base + n] = s_q[a:b]
        dstrel[base:base + n] = (d_q[a:b] - w * WSZ).astype(np.float32)
        ef_perm[base:base + n, :8] = ef_q[a:b]
        ef_perm[base:base + n, 8] = 1.0
    return src_idx, dstrel, ef_perm


def _idx_sb(idx):
    n = idx.shape[0]
    assert n % 16 == 0
    a = np.ascontiguousarray(idx.astype(np.int16).reshape(n // 16, 16).T)
    return np.tile(a, (8, 1)).copy()


# ---------------------------------------------------------------- program
_prog_cache = {}


def _build_program(C):
    import concourse.bass as bass  # noqa: F401
    import concourse.mybir as mybir
    from concourse import bacc
    from concourse.tile import TileContext
    from concourse.masks import make_identity

    f32 = mybir.dt.float32
    f32r = mybir.dt.float32r
    bf16 = mybir.dt.bfloat16
    i16 = mybir.dt.int16
    AF = mybir.ActivationFunctionType
    Alu = mybir.AluOpType

    WC = C // NWIN
    SLAB = 16                      # gather slab, chunks
    assert C % SLAB == 0

    nc = bacc.Bacc("TRN2", num_devices=8)

    xin = nc.declare_dram_parameter("xin", [XIN + 1, NQ], f32, isOutput=False)
    wpre = nc.declare_dram_parameter("wpre", [XIN + 1, H], f32, isOutput=False)
    wedge = nc.declare_dram_parameter("wedge", [9, H], f32, isOutput=False)
    efT = nc.declare_dram_parameter("efT", [9, C * 128], f32, isOutput=False)
    srci = nc.declare_dram_parameter("srci", [128, C * 8], i16, isOutput=False)
    dstrel_d = nc.declare_dram_parameter("dstrel", [128, C], f32, isOutput=False)
    gw1 = nc.declare_dram_parameter("gw1", [L, H, H], f32, isOutput=False)
    gw2 = nc.declare_dram_parameter("gw2", [L, H, H], f32, isOutput=False)
    gb1t = nc.declare_dram_parameter("gb1t", [H, L], f32, isOutput=False)
    gb2t = nc.declare_dram_parameter("gb2t", [H, L], f32, isOutput=False)
    wo1 = nc.declare_dram_parameter("wo1", [CAT, 2 * CAT], f32, isOutput=False)
    wo2 = nc.declare_dram_parameter("wo2", [2 * CAT, CAT], f32, isOutput=False)
    bo1t = nc.declare_dram_parameter("bo1t", [H, 12], f32, isOutput=False)
    bo2t = nc.declare_dram_parameter("bo2t", [H, 6], f32, isOutput=False)
    m1i = nc.declare_dram_parameter("m1i", [128, 48], i16, isOutput=False)
    nsqi = nc.declare_dram_parameter("nsqi", [128, 8], i16, isOutput=False)
    out = nc.declare_dram_parameter("out", [ROWS_D, N], f32, isOutput=True)

    x_rows = nc.dram_tensor("x_rows", [N, H], f32)
    x_ag_in = nc.dram_tensor("x_ag_in", [NQ, H], f32)
    outs_hbm = nc.dram_tensor("outs_hbm", [L, H, NQ], f32r)
    mslice = nc.dram_tensor("mslice", [CAT + 1, NQ], f32)
    mT_all = nc.dram_tensor("mT_all", [8 * (CAT + 1), NQ], f32, addr_space="Shared")

    quads = [[0, 1, 2, 3], [4, 5, 6, 7]]
    allg = [[0, 1, 2, 3, 4, 5, 6, 7]]

    with TileContext(nc) as tc:
        cpool = tc.alloc_tile_pool(name="const", bufs=1)
        ident = cpool.tile([128, 128], f32)
        make_identity(nc, ident[:])
        identr = cpool.tile([128, 128], f32r)
        nc.vector.tensor_copy(identr[:], ident[:])
        iota = cpool.tile([128, WSZ], f32)
        nc.gpsimd.iota(iota[:], pattern=[[1, WSZ]], base=0,
                       channel_multiplier=0,
                       allow_small_or_imprecise_dtypes=True)
        gb1s = cpool.tile([H, L], f32)
        nc.sync.dma_start(out=gb1s[:], in_=gb1t[:])
        gb2s = cpool.tile([H, L], f32)
        nc.sync.dma_start(out=gb2s[:], in_=gb2t[:])
        w1r = cpool.tile([H, L, H], f32r)
        w2r = cpool.tile([H, L, H], f32r)
        xcur = cpool.tile([H, NQ], f32r)
        feat_t = cpool.tile([H, NQ], f32r)
        gpool = tc.alloc_tile_pool(name="grid", bufs=1)
        srct = gpool.tile([128, C * 8], i16)
        nc.sync.dma_start(out=srct[:], in_=srci[:])
        dstrel_t = gpool.tile([128, C], f32)
        nc.sync.dma_start(out=dstrel_t[:], in_=dstrel_d[:])
        e_grid = gpool.tile([128, C, H], bf16)
        aggT = gpool.tile([H, NQ], f32)

        # ---------------- phase A ------------------------------------
        with tc.tile_pool(name="phA", bufs=2) as pa, \
             tc.tile_pool(name="psA", bufs=2, space="PSUM") as ppa:
            wtmp = pa.tile([H, L, H], f32, tag="wtmp")
            nc.sync.dma_start(out=wtmp[:], in_=gw1[:].rearrange("l k m -> k l m"))
            nc.vector.tensor_copy(w1r[:], wtmp[:])
            wtmp2 = pa.tile([H, L, H], f32, tag="wtmp")
            nc.sync.dma_start(out=wtmp2[:], in_=gw2[:].rearrange("l k m -> k l m"))
            nc.vector.tensor_copy(w2r[:], wtmp2[:])

            xinf = pa.tile([XIN + 1, NQ], f32, tag="xinf")
            nc.sync.dma_start(out=xinf[:], in_=xin[:])
            xinr = pa.tile([XIN + 1, NQ], f32r, tag="xinr")
            nc.vector.tensor_copy(xinr[:], xinf[:])
            wpref = pa.tile([XIN + 1, H], f32, tag="wpref")
            nc.sync.dma_start(out=wpref[:], in_=wpre[:])
            wprer = pa.tile([XIN + 1, H], f32r, tag="wprer")
            nc.vector.tensor_copy(wprer[:], wpref[:])
            for nt in range(NQ // 512):
                ps = ppa.tile([H, 512], f32, space="PSUM", tag="psx")
                nc.tensor.matmul(ps[:], lhsT=wprer[:],
                                 rhs=xinr[:, nt * 512:(nt + 1) * 512],
                                 start=True, stop=True)
                nc.vector.tensor_copy(xcur[:, nt * 512:(nt + 1) * 512], ps[:])
            nc.vector.tensor_copy(feat_t[:], xcur[:])

            # x0 rows -> x_ag_in -> AG -> x_rows
            for t in range(NQ // 128):
                pst = ppa.tile([128, 128], f32r, space="PSUM", tag="pst")
                nc.tensor.transpose(out=pst[:],
                                    in_=xcur[:, t * 128:(t + 1) * 128],
                                    identity=identr[:])
                xr = pa.tile([128, H], f32, tag="xr")
                nc.scalar.activation(xr[:], pst[:], AF.Copy)
                nc.sync.dma_start(
                    out=x_ag_in[:].rearrange("(a p) m -> a p m", p=128)[t],
                    in_=xr[:])
            nc.gpsimd.collective_compute(
                "AllGather", Alu.bypass, ins=[x_ag_in[:]], outs=[x_rows[:]],
                replica_groups=quads)

            # edge MLP -> e_grid bf16
            weg = pa.tile([9, H], f32, tag="weg")
            nc.sync.dma_start(out=weg[:], in_=wedge[:])
            wegb = pa.tile([9, H], bf16, tag="wegb")
            nc.vector.tensor_copy(wegb[:], weg[:])
            ES = 16
            for sl in range((C + ES - 1) // ES):
                c0 = sl * ES
                cn = min(ES, C - c0)
                eslab = pa.tile([9, ES * 128], f32, tag="eslab")
                nc.sync.dma_start(out=eslab[:, :cn * 128],
                                  in_=efT[:, c0 * 128:(c0 + cn) * 128])
                eslabb = pa.tile([9, ES * 128], bf16, tag="eslabb")
                nc.vector.tensor_copy(eslabb[:, :cn * 128], eslab[:, :cn * 128])
                for c in range(cn):
                    pse = ppa.tile([128, H], f32, space="PSUM", tag="pse")
                    nc.tensor.matmul(pse[:],
                                     lhsT=eslabb[:, c * 128:(c + 1) * 128],
                                     rhs=wegb[:], start=True, stop=True)
                    nc.scalar.activation(e_grid[:, c0 + c, :], pse[:], AF.Copy)

        # ---------------- phase B: 6 GNN layers ----------------------
        with tc.tile_pool(name="phB", bufs=2) as pb, \
             tc.tile_pool(name="ohB", bufs=3) as pob, \
             tc.tile_pool(name="psB", bufs=2, space="PSUM") as ppb:
            ppm = ppb
            for l in range(L):
                # --- message pass + segment sum
                for w in range(NWIN):
                    psagg = ppb.tile([128, H], f32, space="PSUM", tag="psagg")
                    for cw in range(WC):
                        c = w * WC + cw
                        if c % SLAB == 0:
                            xg = pb.tile([128, SLAB, H], f32, tag="xg")
                            nc.gpsimd.dma_gather(
                                xg[:], x_rows[:],
                                srct[:, c * 8:c * 8 + SLAB * 8],
                                SLAB * 128, SLAB * 128, H, elem_step=H,
                                single_packet=False)
                            tmp = pb.tile([128, SLAB, H], bf16, tag="tmpadd")
                            nc.vector.tensor_tensor(
                                out=tmp[:], in0=xg[:],
                                in1=e_grid[:, c:c + SLAB, :], op=Alu.add)
                            msg = pb.tile([128, SLAB, H], bf16, tag="msg")
                            nc.scalar.activation(msg[:], tmp[:], AF.Relu)
                            oh = pob.tile([128, SLAB, WSZ], bf16, tag="oh")
                            nc.vector.tensor_tensor(
                                out=oh[:],
                                in0=dstrel_t[:, c:c + SLAB].to_broadcast(
                                    [128, SLAB, WSZ]),
                                in1=iota[:].rearrange("p n -> p () n").broadcast_to(
                                    [128, SLAB, WSZ]),
                                op=Alu.is_equal)
                        sc = c % SLAB
                        nc.tensor.matmul(psagg[:], lhsT=oh[:, sc, :],
                                         rhs=msg[:, sc, :],
                                         start=(cw == 0), stop=(cw == WC - 1))
                    # window epilogue: psum [128 nodes, H] -> aggT columns
                    aggn = pb.tile([128, H], f32, tag="aggn")
                    nc.scalar.activation(aggn[:], psagg[:], AF.Copy)
                    psT = ppm.tile([128, 128], f32, space="PSUM", tag="psT")
                    nc.tensor.transpose(out=psT[:], in_=aggn[:], identity=ident[:])
                    nc.vector.tensor_copy(aggT[:, w * WSZ:(w + 1) * WSZ], psT[:])

                # --- node MLP (own 2048 nodes)
                for nt in range(NQ // 512):
                    sl_ = slice(nt * 512, (nt + 1) * 512)
                    ht = pb.tile([H, 512], f32r, tag="ht")
                    nc.vector.tensor_tensor(out=ht[:], in0=xcur[:, sl_],
                                            in1=aggT[:, sl_], op=Alu.add)
                    ps1 = ppm.tile([H, 512], f32, space="PSUM", tag="psmlp")
                    nc.tensor.matmul(ps1[:], lhsT=w1r[:, l, :], rhs=ht[:],
                                     start=True, stop=True)
                    t1 = pb.tile([H, 512], f32r, tag="t1")
                    nc.scalar.activation(t1[:], ps1[:], AF.Relu,
                                         bias=gb1s[:, l:l + 1])
                    ps2 = ppm.tile([H, 512], f32, space="PSUM", tag="psmlp")
                    nc.tensor.matmul(ps2[:], lhsT=w2r[:, l, :], rhs=t1[:],
                                     start=True, stop=True)
                    if l in (1, 3):
                        s0 = pb.tile([H, 512], f32, space="SBUF", tag="s0")
                        nc.scalar.activation(s0[:], ps2[:], AF.Identity,
                                             bias=gb2s[:, l:l + 1])
                        nc.vector.tensor_tensor(out=feat_t[:, sl_], in0=s0[:],
                                                in1=feat_t[:, sl_], op=Alu.add)
                        nc.vector.tensor_relu(xcur[:, sl_], feat_t[:, sl_])
                    else:
                        nc.scalar.activation(xcur[:, sl_], ps2[:], AF.Relu,
                                             bias=gb2s[:, l:l + 1])
                # save layer output (f32r bytes) for phase C
                nc.sync.dma_start(out=outs_hbm[l], in_=xcur[:])
                # x rows AG for next layer
                if l < L - 1:
                    for t in range(NQ // 128):
                        pst = ppm.tile([128, 128], f32r, space="PSUM", tag="psT")
                        nc.tensor.transpose(out=pst[:],
                                            in_=xcur[:, t * 128:(t + 1) * 128],
                                            identity=identr[:])
                        xr = pb.tile([128, H], f32, tag="xr")
                        nc.scalar.activation(xr[:], pst[:], AF.Copy)
                        nc.sync.dma_start(
                            out=x_ag_in[:].rearrange("(a p) m -> a p m", p=128)[t],
                            in_=xr[:])
                    nc.gpsimd.collective_compute(
                        "AllGather", Alu.bypass, ins=[x_ag_in[:]],
                        outs=[x_rows[:]], replica_groups=quads)

        gpool.release()

        # ---------------- phase C: output MLP ------------------------
        with tc.tile_pool(name="phCw", bufs=1) as pcw, \
             tc.tile_pool(name="phC", bufs=2) as pc, \
             tc.tile_pool(name="phCh", bufs=1) as pch, \
             tc.tile_pool(name="psC", bufs=4, space="PSUM") as ppc:
            wo1r = pcw.tile([128, 6, 2 * CAT], f32r, tag="wo1r")
            wo2r = pcw.tile([128, 12, CAT], f32r, tag="wo2r")
            for kc in range(6):
                wt = pc.tile([128, 2 * CAT], f32, tag="wldtmp")
                nc.sync.dma_start(
                    out=wt[:],
                    in_=wo1[:].rearrange("(a p) m -> a p m", p=128)[kc])
                nc.vector.tensor_copy(wo1r[:, kc, :], wt[:])
            for kc in range(12):
                wt = pc.tile([128, CAT], f32, tag="wldtmp")
                nc.sync.dma_start(
                    out=wt[:],
                    in_=wo2[:].rearrange("(a p) m -> a p m", p=128)[kc])
                nc.vector.tensor_copy(wo2r[:, kc, :], wt[:])
            bo1s = pcw.tile([H, 12], f32, tag="bo1s")
            nc.sync.dma_start(out=bo1s[:], in_=bo1t[:])
            bo2s = pcw.tile([H, 6], f32, tag="bo2s")
            nc.sync.dma_start(out=bo2s[:], in_=bo2t[:])
            ones_r = pcw.tile([128, 1], f32r, tag="ones_r")
            onesf = pcw.tile([128, 1], f32, tag="onesf")
            nc.vector.memset(onesf[:], 1.0)
            nc.vector.tensor_copy(ones_r[:], onesf[:])

            nsq_sb = pcw.tile([1, NQ], f32, tag="nsq_sb")
            for nt in range(NQ // 512):
                sl_ = slice(nt * 512, (nt + 1) * 512)
                ne_t = []
                for kc in range(6):
                    nt_t = pc.tile([H, 512], f32r, tag=f"ne{kc}")
                    nc.sync.dma_start(out=nt_t[:], in_=outs_hbm[kc][:, sl_])
                    ne_t.append(nt_t)
                h1 = pch.tile([128, 12, 512], f32r, tag="h1")
                for mt in range(12):
                    ps = ppc.tile([128, 512], f32, space="PSUM", tag="psc")
                    for kc in range(6):
                        nc.tensor.matmul(
                            ps[:], lhsT=wo1r[:, kc, mt * 128:(mt + 1) * 128],
                            rhs=ne_t[kc][:], start=(kc == 0), stop=(kc == 5))
                    nc.scalar.activation(h1[:, mt, :], ps[:], AF.Relu,
                                         bias=bo1s[:, mt:mt + 1])
                sqsum = ppc.tile([1, 512], f32, space="PSUM", tag="sqsum")
                for m2 in range(6):
                    ps = ppc.tile([128, 512], f32, space="PSUM", tag="psc")
                    for kc in range(12):
                        nc.tensor.matmul(
                            ps[:], lhsT=wo2r[:, kc, m2 * 128:(m2 + 1) * 128],
                            rhs=h1[:, kc, :], start=(kc == 0), stop=(kc == 11))
                    mtile = pc.tile([128, 512], f32, tag="mtile")
                    nc.scalar.activation(mtile[:], ps[:], AF.Identity,
                                         bias=bo2s[:, m2:m2 + 1])
                    nc.sync.dma_start(
                        out=mslice[m2 * 128:(m2 + 1) * 128, sl_], in_=mtile[:])
                    sq = pc.tile([128, 512], f32r, tag="sq")
                    nc.vector.tensor_tensor(out=sq[:], in0=mtile[:],
                                            in1=mtile[:], op=Alu.mult)
                    nc.tensor.matmul(sqsum[:], lhsT=ones_r[:], rhs=sq[:],
                                     start=(m2 == 0), stop=(m2 == 5))
                nc.vector.tensor_copy(nsq_sb[:, sl_], sqsum[:])
            nc.sync.dma_start(out=mslice[CAT:CAT + 1, :], in_=nsq_sb[:])
            nc.gpsimd.collective_compute(
                "AllGather", Alu.bypass, ins=[mslice[:]], outs=[mT_all[:]],
                replica_groups=allg)

        # ---------------- phase D: cdist -----------------------------
        with tc.tile_pool(name="phD1", bufs=1) as pd1, \
             tc.tile_pool(name="phD", bufs=2) as pd, \
             tc.tile_pool(name="ohD", bufs=3) as pdd, \
             tc.tile_pool(name="psD", bufs=4, space="PSUM") as ppd:
            m1it = pd1.tile([128, 48], i16, tag="m1it")
            nc.sync.dma_start(out=m1it[:], in_=m1i[:])
            nsqit = pd1.tile([128, 8], i16, tag="nsqit")
            nc.sync.dma_start(out=nsqit[:], in_=nsqi[:])
            vtab = mT_all[:].rearrange("a (b c) -> (a b) c", c=1024)
            m1pre = pd1.tile([128, 6, 1024], f32, tag="m1pre")
            nc.gpsimd.dma_gather(m1pre[:], vtab, m1it[:], CAT, CAT, 1024,
                                 elem_step=1024, single_packet=False)
            m1r = pd1.tile([128, 6, 1024], f32r, tag="m1r")
            nc.vector.tensor_scalar_mul(m1r[:], m1pre[:], -2.0)
            n1all = pd1.tile([128, 1, 1024], f32, tag="n1all")
            nc.gpsimd.dma_gather(n1all[:], vtab, nsqit[:], 128, 128, 1024,
                                 elem_step=1024, single_packet=False)
            n1b = []
            epsb = []
            for b in range(8):
                psn = ppd.tile([128, 128], f32, space="PSUM", tag="psn")
                nc.tensor.transpose(out=psn[:],
                                    in_=n1all[:, 0, b * 128:(b + 1) * 128],
                                    identity=ident[:])
                nb = pd1.tile([128, 1], f32, tag=f"n1b{b}")
                nc.vector.tensor_copy(nb[:], psn[:, 0:1])
                eb = pd1.tile([128, 1], f32, tag=f"epsb{b}")
                nc.vector.tensor_scalar(out=eb[:], in0=nb[:], scalar1=-1.0,
                                        scalar2=EPS, op0=Alu.mult, op1=Alu.add)
                n1b.append(nb)
                epsb.append(eb)
            ones1f = pd1.tile([1, 128], f32, tag="ones1f")
            nc.vector.memset(ones1f[:], 1.0)
            ones1 = pd1.tile([1, 128], f32r, tag="ones1")
            nc.vector.tensor_copy(ones1[:], ones1f[:])

            for s in range(16):
                qs, soff = s // 4, (s % 4) * 512
                base = (4 + qs) * (CAT + 1)
                stf = pd.tile([128, 6, 512], f32, tag="stf")
                nc.sync.dma_start(
                    out=stf[:],
                    in_=mT_all[base:base + CAT, soff:soff + 512].rearrange(
                        "(a p) m -> p a m", p=128))
                st_r = pd.tile([128, 6, 512], f32r, tag="st_r")
                nc.vector.tensor_copy(st_r[:], stf[:])
                n2f = pd.tile([1, 512], f32, tag="n2f")
                nc.sync.dma_start(out=n2f[:],
                                  in_=mT_all[base + CAT:base + CAT + 1,
                                             soff:soff + 512])
                n2r = pd.tile([1, 512], f32r, tag="n2r")
                nc.vector.tensor_copy(n2r[:], n2f[:])
                for b in range(8):
                    psd = ppd.tile([128, 512], f32, space="PSUM", tag="psd")
                    for kc in range(6):
                        nc.tensor.matmul(psd[:],
                                         lhsT=m1r[:, kc, b * 128:(b + 1) * 128],
                                         rhs=st_r[:, kc, :],
                                         start=(kc == 0), stop=False)
                    nc.tensor.matmul(psd[:], lhsT=ones1[:], rhs=n2r[:],
                                     start=False, stop=True)
                    s1 = pdd.tile([128, 512], f32, tag="s1")
                    nc.vector.tensor_scalar(out=s1[:], in0=psd[:],
                                            scalar1=epsb[b][:], scalar2=0.0,
                                            op0=Alu.max, op1=Alu.add)
                    dt_ = pdd.tile([128, 512], f32, tag="dt_")
                    nc.scalar.activation(dt_[:], s1[:], AF.Sqrt,
                                         bias=n1b[b][:])
                    nc.sync.dma_start(
                        out=out[b * 128:(b + 1) * 128, s * 512:(s + 1) * 512],
                        in_=dt_[:])
        cpool.release()

    nc.compile()
    return nc


# ---------------------------------------------------------------- entry
def kernel(**inputs):
    from concourse.bass_utils import run_bass_kernel_spmd

    g1, wc1 = _pack_graph(inputs["edge_index_1"], inputs["e_features1"])
    g2, wc2 = _pack_graph(inputs["edge_index_2"], inputs["e_features2"])
    wc = max(wc1, wc2)
    C = NWIN * wc
    if C % 16 != 0:
        wc += (-wc) % 1  # SLAB=16 divides C iff (16*wc)%16==0, always true
    C = NWIN * wc

    feats = [np.asarray(inputs["features_1"], dtype=np.float32),
             np.asarray(inputs["features_2"], dtype=np.float32)]
    rws = [np.asarray(inputs["RW_1"], dtype=np.float32),
           np.asarray(inputs["RW_2"], dtype=np.float32)]

    wpre_aug = np.vstack([np.asarray(inputs["W_pre"], dtype=np.float32),
                          np.asarray(inputs["b_pre"], dtype=np.float32)[None]])
    wedge_aug = np.vstack([np.asarray(inputs["W_edge"], dtype=np.float32),
                           np.asarray(inputs["b_edge"], dtype=np.float32)[None]])
    gw1 = np.asarray(inputs["gnn_w1"], dtype=np.float32)
    gw2 = np.asarray(inputs["gnn_w2"], dtype=np.float32)
    gb1t = np.ascontiguousarray(np.asarray(inputs["gnn_b1"], np.float32).T)
    gb2t = np.ascontiguousarray(np.asarray(inputs["gnn_b2"], np.float32).T)
    wo1 = np.asarray(inputs["W_out1"], dtype=np.float32)
    wo2 = np.asarray(inputs["W_out2"], dtype=np.float32)
    bo1t = np.ascontiguousarray(
        np.asarray(inputs["b_out1"], np.float32).reshape(12, 128).T)
    bo2t = np.ascontiguousarray(
        np.asarray(inputs["b_out2"], np.float32).reshape(6, 128).T)

    in_maps = []
    for k in range(8):
        g = k // 4          # graph id
        q = k % 4           # quad rank
        src_idx, dstrel, ef_perm = _grid_layout((g1 if g == 0 else g2)[q], wc)
        fx = feats[g][q * NQ:(q + 1) * NQ]
        rx = rws[g][q * NQ:(q + 1) * NQ]
        xin = np.concatenate(
            [fx, rx, np.ones((NQ, 1), np.float32)], axis=1).T.copy()
        # phase D row-block selection: core k -> m1 rows [1024k, 1024k+1024)
        qq, hh = k // 2, k % 2
        m1idx = (2 * ((CAT + 1) * qq + np.arange(CAT)) + hh).astype(np.int16)
        nsqidx = np.full(128, 2 * ((CAT + 1) * qq + CAT) + hh, dtype=np.int16)
        in_maps.append({
            "xin": np.ascontiguousarray(xin),
            "wpre": wpre_aug, "wedge": wedge_aug,
            "efT": np.ascontiguousarray(ef_perm.T),
            "srci": _idx_sb(src_idx),
            "dstrel": np.ascontiguousarray(dstrel.reshape(C, 128).T),
            "gw1": gw1, "gw2": gw2, "gb1t": gb1t, "gb2t": gb2t,
            "wo1": wo1, "wo2": wo2, "bo1t": bo1t, "bo2t": bo2t,
            "m1i": _idx_sb(m1idx),
            "nsqi": _idx_sb(nsqidx),
        })

    if C not in _prog_cache:
        _prog_cache[C] = _build_program(C)
    nc = _prog_cache[C]
    res = run_bass_kernel_spmd(nc, in_maps, list(range(8)), **_run_kwargs)
    global _last_result
    _last_result = res
    return np.vstack([np.asarray(res.results[k]["out"]) for k in range(8)])


_run_kwargs = {}
_last_result = None


# revision 14
# speedup vs baseline: 3258.3203x; 1.0134x over previous
"""Trainium2 Bass kernel for nn_GNOME_42588895707869 (GNN message passing + cdist).

Sharding: cores 0-3 process graph 1, cores 4-7 process graph 2. Within a quad,
core q owns dst-nodes [2048q, 2048(q+1)). Edges are partitioned by dst, sorted,
grouped into 16 windows of 128 nodes, padded to 128-edge chunks. Message
passing per layer: dma_gather of x rows from HBM, relu(x[src]+e) in bf16,
one-hot matmul segment-sum into PSUM (fp32 accumulate), node MLP in fp32r,
quad AllGather of updated x rows. Final cdist: 8-core AllGather of
feature-major m slices; each core's 1024-row block is selected via dma_gather
with per-core index data (keeps the SPMD program core-uniform).
"""
import sys

sys.path.insert(0, "/opt/trn_rl_repo")

import numpy as np  # noqa: E402

N = 8192
H = 128
L = 6
CAT = 768
E = 131072
NQ = 2048          # nodes per core
NWIN = 16          # 128-node windows per core
WSZ = 128
XIN = 64           # features(48) + RW(16)
ROWS_D = 1024      # cdist rows per core
EPS = 1e-12


# ---------------------------------------------------------------- host prep
def _pack_graph(edge_index, e_features):
    src = np.asarray(edge_index[0]).astype(np.int64)
    dst = np.asarray(edge_index[1]).astype(np.int64)
    ef = np.asarray(e_features, dtype=np.float32)
    cores = []
    wchunks = []
    for q in range(4):
        m = (dst // NQ) == q
        s_q, d_q, ef_q = src[m], dst[m] - q * NQ, ef[m]
        order = np.argsort(d_q, kind="stable")
        s_q, d_q, ef_q = s_q[order], d_q[order], ef_q[order]
        cnt = np.bincount(d_q // WSZ, minlength=NWIN)
        cores.append((s_q, d_q, ef_q, cnt))
        wchunks.append(int(np.ceil(cnt / 128).max()))
    return cores, max(wchunks)


def _grid_layout(core, wc):
    s_q, d_q, ef_q, cnt = core
    C = NWIN * wc
    src_idx = np.zeros(C * 128, dtype=np.int16)
    dstrel = np.full(C * 128, -1.0, dtype=np.float32)
    ef_perm = np.zeros((C * 128, 9), dtype=np.float32)
    off = np.concatenate([[0], np.cumsum(cnt)])
    for w in range(NWIN):
        a, b = int(off[w]), int(off[w + 1])
        n = b - a
        base = w * wc * 128
        src_idx[base:base + n] = s_q[a:b]
        dstrel[base:base + n] = (d_q[a:b] - w * WSZ).astype(np.float32)
        ef_perm[base:base + n, :8] = ef_q[a:b]
        ef_perm[base:base + n, 8] = 1.0
    return src_idx, dstrel, ef_perm


def _idx_sb(idx):
    n = idx.shape[0]
    assert n % 16 == 0
    a = np.ascontiguousarray(idx.astype(np.int16).reshape(n // 16, 16).T)
    return np.tile(a, (8, 1)).copy()


# ---------------------------------------------------------------- program
_prog_cache = {}


def _build_program(C):
    import concourse.bass as bass  # noqa: F401
    import concourse.mybir as mybir
    from concourse import bacc
    from concourse.tile import TileContext
    from concourse.masks import make_identity

    f32 = mybir.dt.float32
    f32r = mybir.dt.float32r
    bf16 = mybir.dt.bfloat16
    i16 = mybir.dt.int16
    AF = mybir.ActivationFunctionType
    Alu = mybir.AluOpType

    WC = C // NWIN
    SLAB = 16                      # gather slab, chunks
    assert C % SLAB == 0

    nc = bacc.Bacc("TRN2", num_devices=8)

    xin = nc.declare_dram_parameter("xin", [XIN + 1, NQ], f32, isOutput=False)
    wpre = nc.declare_dram_parameter("wpre", [XIN + 1, H], f32, isOutput=False)
    wedge = nc.declare_dram_parameter("wedge", [9, H], f32, isOutput=False)
    efT = nc.declare_dram_parameter("efT", [9, C * 128], f32, isOutput=False)
    srci = nc.declare_dram_parameter("srci", [128, C * 8], i16, isOutput=False)
    dstrel_d = nc.declare_dram_parameter("dstrel", [128, C], f32, isOutput=False)
    gw1 = nc.declare_dram_parameter("gw1", [L, H, H], f32, isOutput=False)
    gw2 = nc.declare_dram_parameter("gw2", [L, H, H], f32, isOutput=False)
    gb1t = nc.declare_dram_parameter("gb1t", [H, L], f32, isOutput=False)
    gb2t = nc.declare_dram_parameter("gb2t", [H, L], f32, isOutput=False)
    wo1 = nc.declare_dram_parameter("wo1", [CAT, 2 * CAT], f32, isOutput=False)
    wo2 = nc.declare_dram_parameter("wo2", [2 * CAT, CAT], f32, isOutput=False)
    bo1t = nc.declare_dram_parameter("bo1t", [H, 12], f32, isOutput=False)
    bo2t = nc.declare_dram_parameter("bo2t", [H, 6], f32, isOutput=False)
    m1i = nc.declare_dram_parameter("m1i", [128, 48], i16, isOutput=False)
    nsqi = nc.declare_dram_parameter("nsqi", [128, 8], i16, isOutput=False)
    out = nc.declare_dram_parameter("out", [ROWS_D, N], f32, isOutput=True)

    x_rows = nc.dram_tensor("x_rows", [2 * N, H], f32, addr_space="Shared")
    x_ag_in = nc.dram_tensor("x_ag_in", [NQ, H], f32)
    outs_hbm = nc.dram_tensor("outs_hbm", [L, H, NQ], f32r)
    mslice = nc.dram_tensor("mslice", [CAT + 1, NQ], f32)
    mT_all = nc.dram_tensor("mT_all", [8 * (CAT + 1), NQ], f32, addr_space="Shared")

    quads = [[0, 1, 2, 3], [4, 5, 6, 7]]
    allg = [[0, 1, 2, 3, 4, 5, 6, 7]]

    with TileContext(nc) as tc:
        cpool = tc.alloc_tile_pool(name="const", bufs=1)
        ident = cpool.tile([128, 128], f32)
        make_identity(nc, ident[:])
        identr = cpool.tile([128, 128], f32r)
        nc.vector.tensor_copy(identr[:], ident[:])
        iota = cpool.tile([128, WSZ], f32)
        nc.gpsimd.iota(iota[:], pattern=[[1, WSZ]], base=0,
                       channel_multiplier=0,
                       allow_small_or_imprecise_dtypes=True)
        gb1s = cpool.tile([H, L], f32)
        nc.sync.dma_start(out=gb1s[:], in_=gb1t[:])
        gb2s = cpool.tile([H, L], f32)
        nc.sync.dma_start(out=gb2s[:], in_=gb2t[:])
        w1r = cpool.tile([H, L, H], f32r)
        w2r = cpool.tile([H, L, H], f32r)
        xcur = cpool.tile([H, NQ], f32r)
        feat_t = cpool.tile([H, NQ], f32r)
        gpool = tc.alloc_tile_pool(name="grid", bufs=1)
        srct = gpool.tile([128, C * 8], i16)
        nc.sync.dma_start(out=srct[:], in_=srci[:])
        dstrel_t = gpool.tile([128, C], f32)
        nc.sync.dma_start(out=dstrel_t[:], in_=dstrel_d[:])
        e_grid = gpool.tile([128, C, H], bf16)
        aggT = gpool.tile([H, NQ], f32)

        # ---------------- phase A ------------------------------------
        with tc.tile_pool(name="phA", bufs=2) as pa, \
             tc.tile_pool(name="psA", bufs=2, space="PSUM") as ppa:
            wtmp = pa.tile([H, L, H], f32, tag="wtmp")
            nc.sync.dma_start(out=wtmp[:], in_=gw1[:].rearrange("l k m -> k l m"))
            nc.vector.tensor_copy(w1r[:], wtmp[:])
            wtmp2 = pa.tile([H, L, H], f32, tag="wtmp")
            nc.sync.dma_start(out=wtmp2[:], in_=gw2[:].rearrange("l k m -> k l m"))
            nc.vector.tensor_copy(w2r[:], wtmp2[:])

            xinf = pa.tile([XIN + 1, NQ], f32, tag="xinf")
            nc.sync.dma_start(out=xinf[:], in_=xin[:])
            xinr = pa.tile([XIN + 1, NQ], f32r, tag="xinr")
            nc.vector.tensor_copy(xinr[:], xinf[:])
            wpref = pa.tile([XIN + 1, H], f32, tag="wpref")
            nc.sync.dma_start(out=wpref[:], in_=wpre[:])
            wprer = pa.tile([XIN + 1, H], f32r, tag="wprer")
            nc.vector.tensor_copy(wprer[:], wpref[:])
            for nt in range(NQ // 512):
                ps = ppa.tile([H, 512], f32, space="PSUM", tag="psx")
                nc.tensor.matmul(ps[:], lhsT=wprer[:],
                                 rhs=xinr[:, nt * 512:(nt + 1) * 512],
                                 start=True, stop=True)
                nc.vector.tensor_copy(xcur[:, nt * 512:(nt + 1) * 512], ps[:])
            nc.vector.tensor_copy(feat_t[:], xcur[:])

            # x0 rows -> x_ag_in -> AG -> x_rows
            for t in range(NQ // 128):
                pst = ppa.tile([128, 128], f32r, space="PSUM", tag="pst")
                nc.tensor.transpose(out=pst[:],
                                    in_=xcur[:, t * 128:(t + 1) * 128],
                                    identity=identr[:])
                xr = pa.tile([128, H], f32, tag="xr")
                nc.scalar.activation(xr[:], pst[:], AF.Copy)
                nc.sync.dma_start(
                    out=x_ag_in[:].rearrange("(a p) m -> a p m", p=128)[t],
                    in_=xr[:])
            nc.gpsimd.collective_compute(
                "AllGather", Alu.bypass, ins=[x_ag_in[:]], outs=[x_rows[:]],
                replica_groups=allg)

            # edge MLP -> e_grid bf16
            weg = pa.tile([9, H], f32, tag="weg")
            nc.sync.dma_start(out=weg[:], in_=wedge[:])
            wegb = pa.tile([9, H], bf16, tag="wegb")
            nc.vector.tensor_copy(wegb[:], weg[:])
            ES = 16
            for sl in range((C + ES - 1) // ES):
                c0 = sl * ES
                cn = min(ES, C - c0)
                eslab = pa.tile([9, ES * 128], f32, tag="eslab")
                nc.sync.dma_start(out=eslab[:, :cn * 128],
                                  in_=efT[:, c0 * 128:(c0 + cn) * 128])
                eslabb = pa.tile([9, ES * 128], bf16, tag="eslabb")
                nc.vector.tensor_copy(eslabb[:, :cn * 128], eslab[:, :cn * 128])
                for c in range(cn):
                    pse = ppa.tile([128, H], f32, space="PSUM", tag="pse")
                    nc.tensor.matmul(pse[:],
                                     lhsT=eslabb[:, c * 128:(c + 1) * 128],
                                     rhs=wegb[:], start=True, stop=True)
                    nc.scalar.activation(e_grid[:, c0 + c, :], pse[:], AF.Copy)

        # ---------------- phase B: 6 GNN layers ----------------------
        with tc.tile_pool(name="phB", bufs=2) as pb, \
             tc.tile_pool(name="ohB", bufs=3) as pob, \
             tc.tile_pool(name="psB", bufs=2, space="PSUM") as ppb:
            ppm = ppb
            for l in range(L):
                # --- message pass + segment sum
                for w in range(NWIN):
                    psagg = ppb.tile([128, H], f32, space="PSUM", tag="psagg")
                    for cw in range(WC):
                        c = w * WC + cw
                        if c % SLAB == 0:
                            xg = pb.tile([128, SLAB, H], f32, tag="xg")
                            nc.gpsimd.dma_gather(
                                xg[:], x_rows[:],
                                srct[:, c * 8:c * 8 + SLAB * 8],
                                SLAB * 128, SLAB * 128, H, elem_step=H,
                                single_packet=False)
                            tmp = pb.tile([128, SLAB, H], bf16, tag="tmpadd")
                            nc.vector.tensor_tensor(
                                out=tmp[:], in0=xg[:],
                                in1=e_grid[:, c:c + SLAB, :], op=Alu.add)
                            msg = pb.tile([128, SLAB, H], bf16, tag="msg")
                            nc.scalar.activation(msg[:], tmp[:], AF.Relu)
                            oh = pob.tile([128, SLAB, WSZ], bf16, tag="oh")
                            nc.vector.tensor_tensor(
                                out=oh[:],
                                in0=dstrel_t[:, c:c + SLAB].to_broadcast(
                                    [128, SLAB, WSZ]),
                                in1=iota[:].rearrange("p n -> p () n").broadcast_to(
                                    [128, SLAB, WSZ]),
                                op=Alu.is_equal)
                        sc = c % SLAB
                        nc.tensor.matmul(psagg[:], lhsT=oh[:, sc, :],
                                         rhs=msg[:, sc, :],
                                         start=(cw == 0), stop=(cw == WC - 1))
                    # window epilogue: psum [128 nodes, H] -> aggT columns
                    aggn = pb.tile([128, H], f32, tag="aggn")
                    nc.scalar.activation(aggn[:], psagg[:], AF.Copy)
                    psT = ppm.tile([128, 128], f32, space="PSUM", tag="psT")
                    nc.tensor.transpose(out=psT[:], in_=aggn[:], identity=ident[:])
                    nc.vector.tensor_copy(aggT[:, w * WSZ:(w + 1) * WSZ], psT[:])

                # --- node MLP (own 2048 nodes)
                for nt in range(NQ // 512):
                    sl_ = slice(nt * 512, (nt + 1) * 512)
                    ht = pb.tile([H, 512], f32r, tag="ht")
                    nc.vector.tensor_tensor(out=ht[:], in0=xcur[:, sl_],
                                            in1=aggT[:, sl_], op=Alu.add)
                    ps1 = ppm.tile([H, 512], f32, space="PSUM", tag="psmlp")
                    nc.tensor.matmul(ps1[:], lhsT=w1r[:, l, :], rhs=ht[:],
                                     start=True, stop=True)
                    t1 = pb.tile([H, 512], f32r, tag="t1")
                    nc.scalar.activation(t1[:], ps1[:], AF.Relu,
                                         bias=gb1s[:, l:l + 1])
                    ps2 = ppm.tile([H, 512], f32, space="PSUM", tag="psmlp")
                    nc.tensor.matmul(ps2[:], lhsT=w2r[:, l, :], rhs=t1[:],
                                     start=True, stop=True)
                    if l in (1, 3):
                        s0 = pb.tile([H, 512], f32, space="SBUF", tag="s0")
                        nc.scalar.activation(s0[:], ps2[:], AF.Identity,
                                             bias=gb2s[:, l:l + 1])
                        nc.vector.tensor_tensor(out=feat_t[:, sl_], in0=s0[:],
                                                in1=feat_t[:, sl_], op=Alu.add)
                        nc.vector.tensor_relu(xcur[:, sl_], feat_t[:, sl_])
                    else:
                        nc.scalar.activation(xcur[:, sl_], ps2[:], AF.Relu,
                                             bias=gb2s[:, l:l + 1])
                # save layer output (f32r bytes) for phase C
                nc.sync.dma_start(out=outs_hbm[l], in_=xcur[:])
                # x rows AG for next layer
                if l < L - 1:
                    for t in range(NQ // 128):
                        pst = ppm.tile([128, 128], f32r, space="PSUM", tag="psT")
                        nc.tensor.transpose(out=pst[:],
                                            in_=xcur[:, t * 128:(t + 1) * 128],
                                            identity=identr[:])
                        xr = pb.tile([128, H], f32, tag="xr")
                        nc.scalar.activation(xr[:], pst[:], AF.Copy)
                        nc.sync.dma_start(
                            out=x_ag_in[:].rearrange("(a p) m -> a p m", p=128)[t],
                            in_=xr[:])
                    nc.gpsimd.collective_compute(
                        "AllGather", Alu.bypass, ins=[x_ag_in[:]],
                        outs=[x_rows[:]], replica_groups=allg)

        gpool.release()

        # ---------------- phase C: output MLP ------------------------
        with tc.tile_pool(name="phCw", bufs=1) as pcw, \
             tc.tile_pool(name="phC", bufs=2) as pc, \
             tc.tile_pool(name="phCh", bufs=1) as pch, \
             tc.tile_pool(name="psC", bufs=4, space="PSUM") as ppc:
            wo1r = pcw.tile([128, 6, 2 * CAT], f32r, tag="wo1r")
            wo2r = pcw.tile([128, 12, CAT], f32r, tag="wo2r")
            for kc in range(6):
                wt = pc.tile([128, 2 * CAT], f32, tag="wldtmp")
                nc.sync.dma_start(
                    out=wt[:],
                    in_=wo1[:].rearrange("(a p) m -> a p m", p=128)[kc])
                nc.vector.tensor_copy(wo1r[:, kc, :], wt[:])
            for kc in range(12):
                wt = pc.tile([128, CAT], f32, tag="wldtmp")
                nc.sync.dma_start(
                    out=wt[:],
                    in_=wo2[:].rearrange("(a p) m -> a p m", p=128)[kc])
                nc.vector.tensor_copy(wo2r[:, kc, :], wt[:])
            bo1s = pcw.tile([H, 12], f32, tag="bo1s")
            nc.sync.dma_start(out=bo1s[:], in_=bo1t[:])
            bo2s = pcw.tile([H, 6], f32, tag="bo2s")
            nc.sync.dma_start(out=bo2s[:], in_=bo2t[:])
            ones_r = pcw.tile([128, 1], f32r, tag="ones_r")
            onesf = pcw.tile([128, 1], f32, tag="onesf")
            nc.vector.memset(onesf[:], 1.0)
            nc.vector.tensor_copy(ones_r[:], onesf[:])

            nsq_sb = pcw.tile([1, NQ], f32, tag="nsq_sb")
            for nt in range(NQ // 512):
                sl_ = slice(nt * 512, (nt + 1) * 512)
                ne_t = []
                for kc in range(6):
                    nt_t = pc.tile([H, 512], f32r, tag=f"ne{kc}")
                    nc.sync.dma_start(out=nt_t[:], in_=outs_hbm[kc][:, sl_])
                    ne_t.append(nt_t)
                h1 = pch.tile([128, 12, 512], f32r, tag="h1")
                for mt in range(12):
                    ps = ppc.tile([128, 512], f32, space="PSUM", tag="psc")
                    for kc in range(6):
                        nc.tensor.matmul(
                            ps[:], lhsT=wo1r[:, kc, mt * 128:(mt + 1) * 128],
                            rhs=ne_t[kc][:], start=(kc == 0), stop=(kc == 5))
                    nc.scalar.activation(h1[:, mt, :], ps[:], AF.Relu,
                                         bias=bo1s[:, mt:mt + 1])
                sqsum = ppc.tile([1, 512], f32, space="PSUM", tag="sqsum")
                for m2 in range(6):
                    ps = ppc.tile([128, 512], f32, space="PSUM", tag="psc")
                    for kc in range(12):
                        nc.tensor.matmul(
                            ps[:], lhsT=wo2r[:, kc, m2 * 128:(m2 + 1) * 128],
                            rhs=h1[:, kc, :], start=(kc == 0), stop=(kc == 11))
                    mtile = pc.tile([128, 512], f32, tag="mtile")
                    nc.scalar.activation(mtile[:], ps[:], AF.Identity,
                                         bias=bo2s[:, m2:m2 + 1])
                    nc.sync.dma_start(
                        out=mslice[m2 * 128:(m2 + 1) * 128, sl_], in_=mtile[:])
                    sq = pc.tile([128, 512], f32r, tag="sq")
                    nc.vector.tensor_tensor(out=sq[:], in0=mtile[:],
                                            in1=mtile[:], op=Alu.mult)
                    nc.tensor.matmul(sqsum[:], lhsT=ones_r[:], rhs=sq[:],
                                     start=(m2 == 0), stop=(m2 == 5))
                nc.vector.tensor_copy(nsq_sb[:, sl_], sqsum[:])
            nc.sync.dma_start(out=mslice[CAT:CAT + 1, :], in_=nsq_sb[:])
            nc.gpsimd.collective_compute(
                "AllGather", Alu.bypass, ins=[mslice[:]], outs=[mT_all[:]],
                replica_groups=allg)

        # ---------------- phase D: cdist -----------------------------
        with tc.tile_pool(name="phD1", bufs=1) as pd1, \
             tc.tile_pool(name="phD", bufs=2) as pd, \
             tc.tile_pool(name="ohD", bufs=3) as pdd, \
             tc.tile_pool(name="psD", bufs=4, space="PSUM") as ppd:
            m1it = pd1.tile([128, 48], i16, tag="m1it")
            nc.sync.dma_start(out=m1it[:], in_=m1i[:])
            nsqit = pd1.tile([128, 8], i16, tag="nsqit")
            nc.sync.dma_start(out=nsqit[:], in_=nsqi[:])
            vtab = mT_all[:].rearrange("a (b c) -> (a b) c", c=1024)
            m1pre = pd1.tile([128, 6, 1024], f32, tag="m1pre")
            nc.gpsimd.dma_gather(m1pre[:], vtab, m1it[:], CAT, CAT, 1024,
                                 elem_step=1024, single_packet=False)
            m1r = pd1.tile([128, 6, 1024], f32, tag="m1r")
            nc.vector.tensor_scalar_mul(m1r[:], m1pre[:], -2.0)
            n1all = pd1.tile([128, 1, 1024], f32, tag="n1all")
            nc.gpsimd.dma_gather(n1all[:], vtab, nsqit[:], 128, 128, 1024,
                                 elem_step=1024, single_packet=False)
            n1b = []
            epsb = []
            for b in range(8):
                psn = ppd.tile([128, 128], f32, space="PSUM", tag="psn")
                nc.tensor.transpose(out=psn[:],
                                    in_=n1all[:, 0, b * 128:(b + 1) * 128],
                                    identity=ident[:])
                nb = pd1.tile([128, 1], f32, tag=f"n1b{b}")
                nc.vector.tensor_copy(nb[:], psn[:, 0:1])
                eb = pd1.tile([128, 1], f32, tag=f"epsb{b}")
                nc.vector.tensor_scalar(out=eb[:], in0=nb[:], scalar1=-1.0,
                                        scalar2=EPS, op0=Alu.mult, op1=Alu.add)
                n1b.append(nb)
                epsb.append(eb)
            ones1 = pd1.tile([1, 128], f32, tag="ones1")
            nc.vector.memset(ones1[:], 1.0)

            for s in range(16):
                qs, soff = s // 4, (s % 4) * 512
                base = (4 + qs) * (CAT + 1)
                st_r = pd.tile([128, 6, 512], f32, tag="st_r")
                nc.sync.dma_start(
                    out=st_r[:],
                    in_=mT_all[base:base + CAT, soff:soff + 512].rearrange(
                        "(a p) m -> p a m", p=128))
                n2r = pd.tile([1, 512], f32, tag="n2r")
                nc.sync.dma_start(out=n2r[:],
                                  in_=mT_all[base + CAT:base + CAT + 1,
                                             soff:soff + 512])
                for b in range(8):
                    psd = ppd.tile([128, 512], f32, space="PSUM", tag="psd")
                    for kc in range(6):
                        nc.tensor.matmul(psd[:],
                                         lhsT=m1r[:, kc, b * 128:(b + 1) * 128],
                                         rhs=st_r[:, kc, :],
                                         start=(kc == 0), stop=False)
                    nc.tensor.matmul(psd[:], lhsT=ones1[:], rhs=n2r[:],
                                     start=False, stop=True)
                    s1 = pdd.tile([128, 512], f32, tag="s1")
                    nc.vector.tensor_scalar(out=s1[:], in0=psd[:],
                                            scalar1=epsb[b][:], scalar2=0.0,
                                            op0=Alu.max, op1=Alu.add)
                    dt_ = pdd.tile([128, 512], f32, tag="dt_")
                    nc.scalar.activation(dt_[:], s1[:], AF.Sqrt,
                                         bias=n1b[b][:])
                    nc.sync.dma_start(
                        out=out[b * 128:(b + 1) * 128, s * 512:(s + 1) * 512],
                        in_=dt_[:])
        cpool.release()

    nc.compile()
    return nc


# ---------------------------------------------------------------- entry
def kernel(**inputs):
    from concourse.bass_utils import run_bass_kernel_spmd

    g1, wc1 = _pack_graph(inputs["edge_index_1"], inputs["e_features1"])
    g2, wc2 = _pack_graph(inputs["edge_index_2"], inputs["e_features2"])
    wc = max(wc1, wc2)
    C = NWIN * wc
    if C % 16 != 0:
        wc += (-wc) % 1  # SLAB=16 divides C iff (16*wc)%16==0, always true
    C = NWIN * wc

    feats = [np.asarray(inputs["features_1"], dtype=np.float32),
             np.asarray(inputs["features_2"], dtype=np.float32)]
    rws = [np.asarray(inputs["RW_1"], dtype=np.float32),
           np.asarray(inputs["RW_2"], dtype=np.float32)]

    wpre_aug = np.vstack([np.asarray(inputs["W_pre"], dtype=np.float32),
                          np.asarray(inputs["b_pre"], dtype=np.float32)[None]])
    wedge_aug = np.vstack([np.asarray(inputs["W_edge"], dtype=np.float32),
                           np.asarray(inputs["b_edge"], dtype=np.float32)[None]])
    gw1 = np.asarray(inputs["gnn_w1"], dtype=np.float32)
    gw2 = np.asarray(inputs["gnn_w2"], dtype=np.float32)
    gb1t = np.ascontiguousarray(np.asarray(inputs["gnn_b1"], np.float32).T)
    gb2t = np.ascontiguousarray(np.asarray(inputs["gnn_b2"], np.float32).T)
    wo1 = np.asarray(inputs["W_out1"], dtype=np.float32)
    wo2 = np.asarray(inputs["W_out2"], dtype=np.float32)
    bo1t = np.ascontiguousarray(
        np.asarray(inputs["b_out1"], np.float32).reshape(12, 128).T)
    bo2t = np.ascontiguousarray(
        np.asarray(inputs["b_out2"], np.float32).reshape(6, 128).T)

    in_maps = []
    for k in range(8):
        g = k // 4          # graph id
        q = k % 4           # quad rank
        src_idx, dstrel, ef_perm = _grid_layout((g1 if g == 0 else g2)[q], wc)
        if g == 1:
            src_idx = src_idx + np.int16(N)
        fx = feats[g][q * NQ:(q + 1) * NQ]
        rx = rws[g][q * NQ:(q + 1) * NQ]
        xin = np.concatenate(
            [fx, rx, np.ones((NQ, 1), np.float32)], axis=1).T.copy()
        # phase D row-block selection: core k -> m1 rows [1024k, 1024k+1024)
        qq, hh = k // 2, k % 2
        m1idx = (2 * ((CAT + 1) * qq + np.arange(CAT)) + hh).astype(np.int16)
        nsqidx = np.full(128, 2 * ((CAT + 1) * qq + CAT) + hh, dtype=np.int16)
        in_maps.append({
            "xin": np.ascontiguousarray(xin),
            "wpre": wpre_aug, "wedge": wedge_aug,
            "efT": np.ascontiguousarray(ef_perm.T),
            "srci": _idx_sb(src_idx),
            "dstrel": np.ascontiguousarray(dstrel.reshape(C, 128).T),
            "gw1": gw1, "gw2": gw2, "gb1t": gb1t, "gb2t": gb2t,
            "wo1": wo1, "wo2": wo2, "bo1t": bo1t, "bo2t": bo2t,
            "m1i": _idx_sb(m1idx),
            "nsqi": _idx_sb(nsqidx),
        })

    if C not in _prog_cache:
        _prog_cache[C] = _build_program(C)
    nc = _prog_cache[C]
    res = run_bass_kernel_spmd(nc, in_maps, list(range(8)), **_run_kwargs)
    global _last_result
    _last_result = res
    return np.vstack([np.asarray(res.results[k]["out"]) for k in range(8)])


_run_kwargs = {}
_last_result = None
